# revision 13
# baseline (speedup 1.0000x reference)
"""BERT-base forward + per-annotator head on 8 TRN2 NeuronCores.

Data-parallel over batch: each core runs 8 samples x 256 tokens through
the full 12-layer encoder and its per-sample annotator head, producing
(logits, nll) per sample. Host concatenates logits and averages nll.

Layout: activations live in SBUF as [128 part(hidden), 6, T] bf16
(hidden-major), weights are host-pre-transposed to [in, out] so matmul
lhsT tiles DMA contiguously. Matmuls run in bf16 with f32 PSUM
accumulation. Softmax is computed on transposed scores [k, q] so the
reduction is a ones-matmul; the attention-mask additive bias rides the
Exp activation's per-partition bias operand.
"""
import sys
sys.path.insert(0, '/opt/trn_rl_repo')

import numpy as np
import ml_dtypes

import concourse.bass as bass
import concourse.mybir as mybir
from concourse.tile import TileContext
from concourse.bass_utils import run_bass_kernel_spmd
from concourse.masks import make_identity
import concourse.tile_sem_assignment as _tsa
import concourse.tile_utils as _tutils

# Cap DMA-queue semaphore fan-out (keeps per-instruction sync waits low for
# this walrus build) and raise the SBUF budget to trn2's usable 208KB/part.
_tsa.NUM_HWDGE_SEMS = 4
_tsa.NUM_SWDGE_GLOBAL_SEMS = 1
if hasattr(_tutils, 'max_sbuf_usage'):
    _tutils.max_sbuf_usage = 204 * 1024

F32 = mybir.dt.float32
BF16 = mybir.dt.bfloat16
I32 = mybir.dt.int32
AF = mybir.ActivationFunctionType
OP = mybir.AluOpType

VOCAB, HID, LAYERS, HEADS, DH, FF, MAXPOS = 30522, 768, 12, 12, 64, 3072, 512
NUM_ANN, NUM_LABELS = 50, 2
B, S = 64, 256
NCORE = 8
NB = B // NCORE          # samples per core = 8
T = NB * S               # tokens per core = 2048
NSUB = HID // 128        # 6 hidden subtiles
NTT = T // 128           # 16 token tiles
FSUB = FF // 128         # 24

_NC_CACHE = {}


def _split_excess_waits(nc):
    """This walrus build encodes at most 1 sync-wait per instruction.
    Hoist excess waits onto InstNoOps inserted just before the offending
    instruction on the same engine (engine streams are in-order, so the
    hoisted waits still gate it)."""
    n_split = 0
    for f in nc.m.functions:
        for bb in f.blocks:
            insts = bb.instructions
            new = []
            changed = False
            for inst in insts:
                si = inst.sync_info
                if si is not None and si.on_wait and len(si.on_wait) > 1:
                    waits = list(si.on_wait)
                    for w in waits[:-1]:
                        nop = mybir.InstNoOp(
                            name=nc.get_next_instruction_name(),
                            engine=inst.engine, ins=[], outs=[])
                        nop.sync_info = mybir.SyncInfo(on_wait=[w], on_update=[])
                        new.append(nop)
                    inst.sync_info = mybir.SyncInfo(
                        on_wait=[waits[-1]], on_update=list(si.on_update or []))
                    changed = True
                    n_split += 1
                new.append(inst)
            if changed:
                bb.instructions = new
    return n_split


def _build(n_layers=LAYERS):
    nc = bass.Bass()
    ds = bass.ds
    # sem_clear over a wide range emits an InstISA this walrus rejects
    # ("ISA wrong length"); clear in small chunks instead.
    _orig_clear = nc.clear_and_free_semaphores
    def _chunked_clear(sems):
        sems = list(sems)
        for i in range(0, len(sems), 2):
            _orig_clear(sems[i:i + 2])
    nc.clear_and_free_semaphores = _chunked_clear

    # ---- DRAM parameters ----
    wqkvT = nc.declare_dram_parameter("wqkvT", [n_layers, HID, 3 * HID], BF16, isOutput=False)
    waoT = nc.declare_dram_parameter("waoT", [n_layers, HID, HID], BF16, isOutput=False)
    wff1T = nc.declare_dram_parameter("wff1T", [n_layers, HID, FF], BF16, isOutput=False)
    wff2T = nc.declare_dram_parameter("wff2T", [n_layers, FF, HID], BF16, isOutput=False)
    vrows = nc.declare_dram_parameter("vrows", [n_layers, 3, HID], BF16, isOutput=False)
    bias_pt = nc.declare_dram_parameter("bias_pt", [n_layers, 128, 60], F32, isOutput=False)
    emb_pt = nc.declare_dram_parameter("emb_pt", [128, 12], F32, isOutput=False)
    word_emb = nc.declare_dram_parameter("word_emb", [VOCAB, HID], F32, isOutput=False)
    posty = nc.declare_dram_parameter("posty", [T, HID], F32, isOutput=False)
    ids_pt = nc.declare_dram_parameter("ids_pt", [128, NTT], I32, isOutput=False)
    bm_pt = nc.declare_dram_parameter("bm_pt", [128, NTT], F32, isOutput=False)
    head_w100 = nc.declare_dram_parameter("head_w100", [NUM_ANN * NUM_LABELS, HID], F32, isOutput=False)
    head_b50 = nc.declare_dram_parameter("head_b50", [NUM_ANN, NUM_LABELS], F32, isOutput=False)
    hidx0 = nc.declare_dram_parameter("hidx0", [NB, 1], I32, isOutput=False)
    hidx1 = nc.declare_dram_parameter("hidx1", [NB, 1], I32, isOutput=False)
    aidx = nc.declare_dram_parameter("aidx", [NB, 1], I32, isOutput=False)
    lab = nc.declare_dram_parameter("lab", [NB, 1], F32, isOutput=False)
    out_ext = nc.declare_dram_parameter("out", [NB, 3], F32, isOutput=True)

    from contextlib import ExitStack
    with ExitStack() as _ctx:
        tc = _ctx.enter_context(TileContext(nc))
        pc = _ctx.enter_context(tc.tile_pool(name="const", bufs=1))
        pwq = _ctx.enter_context(tc.tile_pool(name="wq", bufs=1))
        pwao = _ctx.enter_context(tc.tile_pool(name="wao", bufs=1))
        pwf1 = _ctx.enter_context(tc.tile_pool(name="wf1", bufs=1))
        pwf2 = _ctx.enter_context(tc.tile_pool(name="wf2", bufs=1))
        pxp = _ctx.enter_context(tc.tile_pool(name="xp", bufs=1))
        pqv = _ctx.enter_context(tc.tile_pool(name="qv", bufs=5))
        phh = _ctx.enter_context(tc.tile_pool(name="hh", bufs=1))
        psq = _ctx.enter_context(tc.tile_pool(name="sq", bufs=2))
        ppr = _ctx.enter_context(tc.tile_pool(name="pr", bufs=2))
        ppl = _ctx.enter_context(tc.tile_pool(name="pl", bufs=1))
        pst = _ctx.enter_context(tc.tile_pool(name="st", bufs=4))
        poo = _ctx.enter_context(tc.tile_pool(name="oo", bufs=1))
        psb = _ctx.enter_context(tc.tile_pool(name="psb", bufs=3, space="PSUM"))
        ps2 = _ctx.enter_context(tc.tile_pool(name="ps2", bufs=3, space="PSUM"))
        pstt = _ctx.enter_context(tc.tile_pool(name="pst2", bufs=2, space="PSUM"))
        if True:
            # ---- constants ----
            ones_cbf = pc.tile([128, 1], BF16)      # column of ones (lhsT for col-sum)
            nc.vector.memset(ones_cbf[:], 1.0)
            ones_cf = pc.tile([128, 1], F32)
            nc.vector.memset(ones_cf[:], 1.0)
            ones_rf = pc.tile([1, 128], F32)        # row of ones (lhsT for bcast)
            nc.vector.memset(ones_rf[:], 1.0)
            ones_rbf = pc.tile([1, 128], BF16)
            nc.vector.memset(ones_rbf[:], 1.0)
            ones_tbf = pc.tile([1, S], BF16)        # rhs of per-partition bias matmul
            nc.vector.memset(ones_tbf[:], 1.0)
            idf = pc.tile([128, 128], F32)
            idb = pc.tile([128, 128], BF16)
            import os as _os2
            if _os2.environ.get("K_NOIDENT"):
                nc.vector.memset(idf[:], 0.0)
                nc.vector.memset(idb[:], 0.0)
            else:
                make_identity(nc, idf[:])
                make_identity(nc, idb[:])

            emb_sb = pc.tile([128, 12], F32)
            nc.sync.dma_start(emb_sb[:], emb_pt[:])
            ids_sb = pc.tile([128, NTT], I32)
            nc.sync.dma_start(ids_sb[:], ids_pt[:])
            bm_sb = pc.tile([128, NTT], F32)
            nc.sync.dma_start(bm_sb[:], bm_pt[:])


            def _recip(out_ap, in_ap):
                import os as _os3
                if _os3.environ.get("K_NORECIP"):
                    nc.vector.tensor_copy(out=out_ap, in_=in_ap)
                else:
                    nc.vector.reciprocal(out_ap, in_ap)
            # persistent activation [hid(6x128), T] bf16
            x = pxp.tile([128, NSUB, T], BF16)

            # ---------- layer norm over hidden (partition axis), in place on x ----------
            def layer_norm_cols(c0, w, g_col_ap, b_col_ap):
                """LN of x[:, :, c0:c0+w] in place. g/b: [128, NSUB]-style slices
                indexed per subtile via the passed lambdas."""
                sq = psq.tile([128, NSUB, S], F32, tag="sq")
                sums = pstt.tile([1, S], F32, tag="stat", name="sums")[:, :w]
                sqs = pstt.tile([1, S], F32, tag="stat", name="sqs")[:, :w]
                for j in range(NSUB):
                    nc.vector.tensor_tensor(out=sq[:, j, :w], in0=x[:, j, c0:c0 + w],
                                            in1=x[:, j, c0:c0 + w], op=OP.mult)
                for j in range(NSUB):
                    nc.tensor.matmul(out=sums[:], lhsT=ones_cbf[:], rhs=x[:, j, c0:c0 + w],
                                     start=(j == 0), stop=(j == NSUB - 1))
                for j in range(NSUB):
                    nc.tensor.matmul(out=sqs[:], lhsT=ones_cf[:], rhs=sq[:, j, :w],
                                     start=(j == 0), stop=(j == NSUB - 1))
                negmu = pst.tile([1, S], F32, tag="st", name="negmu")[:, :w]
                nc.scalar.activation(negmu, sums, AF.Copy, bias=0.0, scale=-1.0 / HID)
                musq = pst.tile([1, S], F32, tag="st", name="musq")[:, :w]
                nc.scalar.activation(musq, negmu, AF.Square)
                vt = pst.tile([1, S], F32, tag="st", name="vt")[:, :w]
                nc.scalar.activation(vt, sqs, AF.Copy, bias=1e-12, scale=1.0 / HID)
                nc.vector.tensor_tensor(out=vt, in0=vt, in1=musq, op=OP.subtract)
                rec = pst.tile([1, S], F32, tag="st", name="rec")[:, :w]
                _recip(rec, vt)
                rstd = pst.tile([1, S], F32, tag="st", name="rstd")[:, :w]
                nc.scalar.activation(rstd, rec, AF.Sqrt)
                nm_bc = psb.tile([128, S], F32, tag="big", name="nm_bc")[:, :w]
                nc.tensor.matmul(out=nm_bc, lhsT=ones_rf[:], rhs=negmu, start=True, stop=True)
                rs_bc = psb.tile([128, S], F32, tag="big", name="rs_bc")[:, :w]
                nc.tensor.matmul(out=rs_bc, lhsT=ones_rf[:], rhs=rstd, start=True, stop=True)
                for j in range(NSUB):
                    nc.vector.tensor_tensor(out=sq[:, j, :w], in0=x[:, j, c0:c0 + w],
                                            in1=nm_bc, op=OP.add)
                    nc.vector.tensor_tensor(out=sq[:, j, :w], in0=sq[:, j, :w],
                                            in1=rs_bc, op=OP.mult)
                    nc.vector.tensor_scalar(out=x[:, j, c0:c0 + w], in0=sq[:, j, :w],
                                            scalar1=g_col_ap(j), scalar2=b_col_ap(j),
                                            op0=OP.mult, op1=OP.add)

            # ---------- embedding: gather + posty + transpose into x ----------
            for it in range(NTT):
                g_t = psq.tile([128, HID], F32, tag="sq")
                if __import__("os").environ.get("K_NOGATHER"):
                    nc.sync.dma_start(g_t[:], word_emb[it * 128:(it + 1) * 128, :])
                else:
                    nc.gpsimd.indirect_dma_start(
                        out=g_t[:], out_offset=None, in_=word_emb[:],
                        in_offset=bass.IndirectOffsetOnAxis(ap=ids_sb[:, it:it + 1], axis=0))
                pt_t = psq.tile([128, HID], F32, tag="sq")
                nc.sync.dma_start(pt_t[:], posty[it * 128:(it + 1) * 128, :])
                nc.vector.tensor_tensor(out=g_t[:], in0=g_t[:], in1=pt_t[:], op=OP.add)
                for j in range(NSUB):
                    tp = ps2.tile([128, 128], F32, tag="ctx2", name="tp")
                    nc.tensor.transpose(tp[:], g_t[:, j * 128:(j + 1) * 128], idf[:])
                    nc.vector.tensor_copy(out=x[:, j, it * 128:(it + 1) * 128], in_=tp[:])
            for c in range(T // S):
                layer_norm_cols(c * S, S,
                                lambda j: emb_sb[:, j:j + 1],
                                lambda j: emb_sb[:, 6 + j:7 + j])

            # ---------- encoder layers ----------
            import os as _os, contextlib as _cl
            if _os.environ.get("K_STATIC"):
                _layer_cm = _cl.nullcontext(0)
            else:
                _layer_cm = tc.For_i(0, n_layers, name="layer")
            with _layer_cm as l:
                wq = pwq.tile([128, NSUB, 3 * HID], BF16)
                nc.sync.dma_start(wq[:], wqkvT[ds(l, 1)].rearrange(
                    "z (hs p) o -> p (z hs) o", p=128))
                wao = pwao.tile([128, NSUB, HID], BF16)
                nc.sync.dma_start(wao[:], waoT[ds(l, 1)].rearrange(
                    "z (hs p) o -> p (z hs) o", p=128))
                wf1 = pwf1.tile([128, NSUB, FF], BF16)
                nc.sync.dma_start(wf1[:], wff1T[ds(l, 1)].rearrange(
                    "z (hs p) o -> p (z hs) o", p=128))
                wf2 = pwf2.tile([128, FSUB, HID], BF16)
                nc.sync.dma_start(wf2[:], wff2T[ds(l, 1)].rearrange(
                    "z (fs p) o -> p (z fs) o", p=128))
                bl = ppl.tile([128, 60], F32, tag="bl")
                nc.sync.dma_start(bl[:], bias_pt[ds(l, 1)].rearrange("z p c -> (z p) c"))
                rows_bf = ppl.tile([1, 3, HID], BF16, tag="rb")
                nc.sync.dma_start(rows_bf[:], vrows[ds(l, 1)].rearrange("z r o -> z (r o)"))

                for s in range(NB):
                    cs = s * S
                    # ---- QKV ----
                    q_sb = pqv.tile([128, NSUB, S], BF16, tag="qv")
                    k_sb = pqv.tile([128, NSUB, S], BF16, tag="qv")
                    vT_sb = pqv.tile([128, 2, HID], BF16, tag="qv")
                    # Q,K: out[o,t] tiles, per-partition bias from bl
                    for ot in range(12):
                        ps = psb.tile([128, S], F32, tag="big")
                        for j in range(NSUB):
                            nc.tensor.matmul(out=ps[:], lhsT=wq[:, j, ot * 128:(ot + 1) * 128],
                                             rhs=x[:, j, cs:cs + S],
                                             start=(j == 0), stop=(j == NSUB - 1))
                        dst = q_sb[:, ot, :] if ot < NSUB else k_sb[:, ot - NSUB, :]
                        nc.vector.tensor_scalar_add(out=dst, in0=ps[:], scalar1=bl[:, ot:ot + 1])
                    # V transposed: out[t, dv] via x-stationary matmuls + bias matmul
                    for tt in range(2):
                        for c2 in range(2):
                            o0 = c2 * 384
                            ps = psb.tile([128, 384], F32, tag="big", name="psv")
                            for j in range(NSUB):
                                nc.tensor.matmul(
                                    out=ps,
                                    lhsT=x[:, j, cs + tt * 128:cs + (tt + 1) * 128],
                                    rhs=wq[:, j, 2 * HID + o0:2 * HID + o0 + 384],
                                    start=(j == 0), stop=False)
                            nc.tensor.matmul(out=ps, lhsT=ones_rbf[:],
                                             rhs=rows_bf[:, 0, o0:o0 + 384],
                                             start=False, stop=True)
                            nc.vector.tensor_copy(out=vT_sb[:, tt, o0:o0 + 384], in_=ps)
                    # ---- attention ----
                    ctx_sb = pqv.tile([128, NSUB, S], BF16, tag="qv")
                    for h in range(HEADS):
                        hb = (h % 2) * 64
                        hj = h // 2
                        sc = psb.tile([128, 2, S], F32, tag="big")
                        for j in range(2):
                            nc.tensor.matmul(
                                out=sc[:, j, :],
                                lhsT=k_sb[hb:hb + 64, hj, j * 128:(j + 1) * 128],
                                rhs=q_sb[hb:hb + 64, hj, :],
                                start=True, stop=True)
                        pr = ppr.tile([128, 2, S], BF16, tag="pr")
                        for j in range(2):
                            nc.scalar.activation(pr[:, j, :], sc[:, j, :], AF.Exp,
                                                 bias=bm_sb[:, 2 * s + j:2 * s + j + 1],
                                                 scale=0.125)
                        sums = pstt.tile([1, S], F32, tag="stat", name="sums")
                        ctxp = ps2.tile([64, S], F32, tag="ctx2", name="ctxp")
                        for j in range(2):
                            nc.tensor.matmul(out=sums[:], lhsT=ones_cbf[:], rhs=pr[:, j, :],
                                             start=(j == 0), stop=(j == 1))
                            nc.tensor.matmul(out=ctxp[:],
                                             lhsT=vT_sb[:, j, h * 64:(h + 1) * 64],
                                             rhs=pr[:, j, :],
                                             start=(j == 0), stop=(j == 1))
                        rec = pst.tile([1, S], F32, tag="st")
                        _recip(rec[:], sums[:])
                        bc = ps2.tile([64, S], F32, tag="ctx2", name="bc")
                        nc.tensor.matmul(out=bc[:], lhsT=ones_rf[:, :64], rhs=rec[:],
                                         start=True, stop=True)
                        bcs = pst.tile([64, S], F32, tag="bcs", name="bcs")
                        nc.vector.tensor_copy(out=bcs[:], in_=bc[:])
                        nc.vector.tensor_tensor(out=ctx_sb[hb:hb + 64, hj, :],
                                                in0=ctxp[:], in1=bcs[:], op=OP.mult)
                    # ---- attention output + residual ----
                    for ot in range(NSUB):
                        ps = psb.tile([128, S], F32, tag="big")
                        for j in range(NSUB):
                            nc.tensor.matmul(out=ps[:], lhsT=wao[:, j, ot * 128:(ot + 1) * 128],
                                             rhs=ctx_sb[:, j, :],
                                             start=(j == 0), stop=False)
                        nc.tensor.matmul(out=ps[:], lhsT=rows_bf[:, 1, ot * 128:(ot + 1) * 128],
                                         rhs=ones_tbf[:], start=False, stop=True)
                        nc.vector.tensor_tensor(out=x[:, ot, cs:cs + S], in0=ps[:],
                                                in1=x[:, ot, cs:cs + S], op=OP.add)
                    layer_norm_cols(cs, S,
                                    lambda j: bl[:, 36 + j:37 + j],
                                    lambda j: bl[:, 42 + j:43 + j])
                    # ---- FFN ----
                    h_sb = phh.tile([128, FSUB, S], BF16, tag="hh")
                    for ft in range(FSUB):
                        ps = psb.tile([128, S], F32, tag="big")
                        for j in range(NSUB):
                            nc.tensor.matmul(out=ps[:], lhsT=wf1[:, j, ft * 128:(ft + 1) * 128],
                                             rhs=x[:, j, cs:cs + S],
                                             start=(j == 0), stop=(j == NSUB - 1))
                        nc.scalar.activation(h_sb[:, ft, :], ps[:], AF.Gelu,
                                             bias=bl[:, 12 + ft:13 + ft], scale=1.0)
                    for ot in range(NSUB):
                        ps = psb.tile([128, S], F32, tag="big")
                        for j in range(FSUB):
                            nc.tensor.matmul(out=ps[:], lhsT=wf2[:, j, ot * 128:(ot + 1) * 128],
                                             rhs=h_sb[:, j, :],
                                             start=(j == 0), stop=False)
                        nc.tensor.matmul(out=ps[:], lhsT=rows_bf[:, 2, ot * 128:(ot + 1) * 128],
                                         rhs=ones_tbf[:], start=False, stop=True)
                        nc.vector.tensor_tensor(out=x[:, ot, cs:cs + S], in0=ps[:],
                                                in1=x[:, ot, cs:cs + S], op=OP.add)
                    layer_norm_cols(cs, S,
                                    lambda j: bl[:, 48 + j:49 + j],
                                    lambda j: bl[:, 54 + j:55 + j])

            # ---------- per-annotator head ----------
            clsT = poo.tile([NB, HID], F32)
            for j in range(NSUB):
                tp = ps2.tile([128, 128], BF16, tag="ctx2", name="tpc")[:NB, :]
                x_cls = x[:, j].rearrange("p (s t) -> p s t", t=S)[:, :, 0]
                nc.tensor.transpose(tp, x_cls, idb[:])
                nc.vector.tensor_copy(out=clsT[:, j * 128:(j + 1) * 128], in_=tp)
            hw0 = poo.tile([NB, HID], F32)
            hw1 = poo.tile([NB, HID], F32)
            hb_sb = poo.tile([NB, 2], F32)
            idx0 = poo.tile([NB, 1], I32)
            idx1 = poo.tile([NB, 1], I32)
            idxa = poo.tile([NB, 1], I32)
            nc.sync.dma_start(idx0[:], hidx0[:])
            nc.sync.dma_start(idx1[:], hidx1[:])
            nc.sync.dma_start(idxa[:], aidx[:])
            nc.gpsimd.indirect_dma_start(out=hw0[:], out_offset=None, in_=head_w100[:],
                                         in_offset=bass.IndirectOffsetOnAxis(ap=idx0[:, :1], axis=0))
            nc.gpsimd.indirect_dma_start(out=hw1[:], out_offset=None, in_=head_w100[:],
                                         in_offset=bass.IndirectOffsetOnAxis(ap=idx1[:, :1], axis=0))
            nc.gpsimd.indirect_dma_start(out=hb_sb[:], out_offset=None, in_=head_b50[:],
                                         in_offset=bass.IndirectOffsetOnAxis(ap=idxa[:, :1], axis=0))
            lab_sb = poo.tile([NB, 1], F32)
            nc.sync.dma_start(lab_sb[:], lab[:])

            lgt = poo.tile([NB, 4], F32)
            nc.vector.tensor_tensor(out=hw0[:], in0=clsT[:], in1=hw0[:], op=OP.mult)
            nc.scalar.activation(hw0[:], hw0[:], AF.Copy, accum_out=lgt[:, 0:1])
            nc.vector.tensor_tensor(out=hw1[:], in0=clsT[:], in1=hw1[:], op=OP.mult)
            nc.scalar.activation(hw1[:], hw1[:], AF.Copy, accum_out=lgt[:, 1:2])
            nc.vector.tensor_tensor(out=lgt[:, 0:2], in0=lgt[:, 0:2], in1=hb_sb[:], op=OP.add)
            # loss pieces: m = max(l0,l1); se = sum exp(l - m); nll = log(se)+m - l[label]
            m = poo.tile([NB, 4], F32)
            nc.vector.tensor_tensor(out=m[:, 0:1], in0=lgt[:, 0:1], in1=lgt[:, 1:2], op=OP.max)
            nc.vector.tensor_scalar_mul(out=m[:, 1:2], in0=m[:, 0:1], scalar1=-1.0)
            e = poo.tile([NB, 2], F32)
            se = poo.tile([NB, 4], F32)
            nc.scalar.activation(e[:], lgt[:, 0:2], AF.Exp, bias=m[:, 1:2], scale=1.0,
                                 accum_out=se[:, 0:1])
            nc.scalar.activation(se[:, 1:2], se[:, 0:1], AF.Ln)
            nc.vector.tensor_tensor(out=se[:, 2:3], in0=se[:, 1:2], in1=m[:, 0:1], op=OP.add)
            # sel = l0 + lab*(l1-l0); nll = logZ - sel
            nc.vector.tensor_tensor(out=m[:, 2:3], in0=lgt[:, 1:2], in1=lgt[:, 0:1], op=OP.subtract)
            nc.vector.tensor_scalar_mul(out=m[:, 3:4], in0=m[:, 2:3], scalar1=lab_sb[:, 0:1])
            nc.vector.tensor_tensor(out=m[:, 3:4], in0=m[:, 3:4], in1=lgt[:, 0:1], op=OP.add)
            nc.vector.tensor_tensor(out=lgt[:, 2:3], in0=se[:, 2:3], in1=m[:, 3:4], op=OP.subtract)
            out_sb = poo.tile([NB, 3], F32)
            nc.vector.tensor_copy(out=out_sb[:], in_=lgt[:, 0:3])
            nc.sync.dma_start(out_ext[:], out_sb[:])

    _split_excess_waits(nc)
    return nc


def _host_prep(inputs):
    """Shard + re-layout inputs for the 8 cores. Returns in_maps list."""
    f32 = np.float32
    ids = np.asarray(inputs["input_ids"]).astype(np.int32)          # [B, S]
    mask = np.asarray(inputs["attention_mask"]).astype(f32)         # [B, S]
    tti = np.asarray(inputs["token_type_ids"]).astype(np.int32)     # [B, S]
    ann = np.asarray(inputs["annotator_idx"]).astype(np.int32)      # [B]
    labels = np.asarray(inputs["labels"]).astype(np.int32)          # [B]

    bf = ml_dtypes.bfloat16
    wqkvT = np.ascontiguousarray(np.asarray(inputs["qkv_w"]).transpose(0, 2, 1)).astype(bf)
    waoT = np.ascontiguousarray(np.asarray(inputs["ao_w"]).transpose(0, 2, 1)).astype(bf)
    wff1T = np.ascontiguousarray(np.asarray(inputs["ff1_w"]).transpose(0, 2, 1)).astype(bf)
    wff2T = np.ascontiguousarray(np.asarray(inputs["ff2_w"]).transpose(0, 2, 1)).astype(bf)

    qkv_b = np.asarray(inputs["qkv_b"]).astype(f32)
    ao_b = np.asarray(inputs["ao_b"]).astype(f32)
    ff1_b = np.asarray(inputs["ff1_b"]).astype(f32)
    ff2_b = np.asarray(inputs["ff2_b"]).astype(f32)
    ln1_g = np.asarray(inputs["ln1_g"]).astype(f32)
    ln1_b = np.asarray(inputs["ln1_b"]).astype(f32)
    ln2_g = np.asarray(inputs["ln2_g"]).astype(f32)
    ln2_b = np.asarray(inputs["ln2_b"]).astype(f32)

    vrows = np.ascontiguousarray(np.stack([qkv_b[:, 2 * HID:], ao_b, ff2_b], axis=1)).astype(bf)  # [L, 3, 768]
    bias_pt = np.empty((LAYERS, 128, 60), f32)
    bias_pt[:, :, 0:12] = qkv_b[:, :2 * HID].reshape(LAYERS, 12, 128).transpose(0, 2, 1)
    bias_pt[:, :, 12:36] = ff1_b.reshape(LAYERS, 24, 128).transpose(0, 2, 1)
    bias_pt[:, :, 36:42] = ln1_g.reshape(LAYERS, 6, 128).transpose(0, 2, 1)
    bias_pt[:, :, 42:48] = ln1_b.reshape(LAYERS, 6, 128).transpose(0, 2, 1)
    bias_pt[:, :, 48:54] = ln2_g.reshape(LAYERS, 6, 128).transpose(0, 2, 1)
    bias_pt[:, :, 54:60] = ln2_b.reshape(LAYERS, 6, 128).transpose(0, 2, 1)
    bias_pt = np.ascontiguousarray(bias_pt)

    emb_pt = np.empty((128, 12), f32)
    emb_pt[:, 0:6] = np.asarray(inputs["emb_ln_g"]).astype(f32).reshape(6, 128).T
    emb_pt[:, 6:12] = np.asarray(inputs["emb_ln_b"]).astype(f32).reshape(6, 128).T

    word = np.ascontiguousarray(np.asarray(inputs["word_emb"]).astype(f32))
    pos = np.asarray(inputs["pos_emb"]).astype(f32)[:S]              # [S, 768]
    type_emb = np.asarray(inputs["type_emb"]).astype(f32)            # [2, 768]
    head_w100 = np.ascontiguousarray(
        np.asarray(inputs["head_w"]).astype(f32).reshape(NUM_ANN * NUM_LABELS, HID))
    head_b50 = np.ascontiguousarray(np.asarray(inputs["head_b"]).astype(f32))

    in_maps = []
    for c in range(NCORE):
        sl = slice(c * NB, (c + 1) * NB)
        ids_c = ids[sl].reshape(T)
        mask_c = mask[sl].reshape(T)
        tti_c = tti[sl].reshape(T)
        posty = np.ascontiguousarray(
            np.tile(pos, (NB, 1)) + type_emb[tti_c])                 # [T, 768]
        ids_pt = np.ascontiguousarray(ids_c.reshape(NTT, 128).T)     # [128, NTT]
        bm_pt = np.ascontiguousarray(
            (-10000.0 * (1.0 - mask_c)).reshape(NTT, 128).T).astype(f32)
        ann_c = ann[sl]
        in_maps.append({
            "wqkvT": wqkvT, "waoT": waoT, "wff1T": wff1T, "wff2T": wff2T,
            "vrows": vrows, "bias_pt": bias_pt, "emb_pt": emb_pt,
            "word_emb": word, "posty": posty, "ids_pt": ids_pt, "bm_pt": bm_pt,
            "head_w100": head_w100, "head_b50": head_b50,
            "hidx0": (2 * ann_c).reshape(NB, 1).astype(np.int32),
            "hidx1": (2 * ann_c + 1).reshape(NB, 1).astype(np.int32),
            "aidx": ann_c.reshape(NB, 1).astype(np.int32),
            "lab": labels[sl].reshape(NB, 1).astype(f32),
        })
    return in_maps


def kernel(**inputs):
    if "nc" not in _NC_CACHE:
        _NC_CACHE["nc"] = _build()
    nc = _NC_CACHE["nc"]
    in_maps = _host_prep(inputs)
    res = run_bass_kernel_spmd(nc, in_maps, core_ids=list(range(NCORE)))
    outs = np.concatenate([res.results[c]["out"] for c in range(NCORE)], axis=0)  # [64, 3]
    logits = np.ascontiguousarray(outs[:, 0:2]).astype(np.float32)
    loss = np.float32(outs[:, 2].mean())
    return loss, logits
